# revision 80
# baseline (speedup 1.0000x reference)
"""Cross-attention layer (B=2, N1=N2=2048, 16 heads x 64, fp32) on 8 trn2 cores.

Sharding: core c = (batch b=c//4, query-row slice r=c%4 of 512 rows).
K/V projections are replicated within a batch group (no collectives needed);
every other stage is fully data-parallel.  Output is a pure host-side concat.

Schedule (cost-model driven): the exp stream on ACT and the matmul stream on
PE are the two big engine budgets, so the K projection for head-pairs 1..7 and
the whole V projection are interleaved INTO the attention loop where the PE
would otherwise wait on ACT.  All matmul inputs are bf16 (host pre-casts;
matmul billing is identical to fp32r but DMA/SBUF halve), accumulation stays
fp32 in PSUM, LayerNorm runs fp32 stats over a bf16 staging of Z.

Device algorithm per core:
  QT[e,q] = (Wq^T q^T + bq)          bf16, e on partitions
  KT[e,k] = (Wk^T k^T + bk)          bf16 (head-pair ec emitted just-in-time)
  V'[k,(h,d|1)] = v^T^T Wv, ones column per head (softmax denominator trick)
  per head h:  ST[k,q] = KT_h^T QT_h         (16 k-chunks, 2 chunks/PSUM tile)
               ET = exp(ST/8)                 (ACT, 1024-wide reads)
               UT'[65,q] += V'_h^T ET         (PSUM accumulate)
      row 64 of UT' is the softmax denominator s[q]; 1/s broadcast across
      partitions with a stride-0 SBUF->SBUF DMA, then one DVE multiply.
  Z[q,n] = sum over head-pairs UTpair^T Wo-rows + (bv@Wo + bo)
  out    = LayerNorm(Z) * gamma + beta
"""

import sys

for _p in ("/opt/trn_rl_repo",):
    if _p not in sys.path:
        sys.path.insert(0, _p)

import ml_dtypes
import numpy as np

import concourse.bass as bass
import concourse.mybir as mybir
import concourse.tile as tile
from concourse import bacc
from concourse.bass_utils import run_bass_kernel_spmd

F32 = mybir.dt.float32
F32R = mybir.dt.float32r
BF16 = mybir.dt.bfloat16
AF = mybir.ActivationFunctionType
OP = mybir.AluOpType
NPBF16 = np.dtype(ml_dtypes.bfloat16)

B = 2
N1 = 2048
N2 = 2048
CQ = 512  # query feature dim
CKV = 512  # key/value feature dim
E = 1024  # embed dim
H = 16  # heads
D = 64  # head dim
EPS = 1e-5
NCORES = 8
QSL = N1 * B // NCORES  # 512 query rows per core
SCALE = float(D) ** -0.5


def _bcast_rows(src_ap, nparts=128):
    """Broadcast a 1-D DRAM AP across partitions (step-0 partition dim)."""
    return bass.AP(
        tensor=src_ap.tensor, offset=src_ap.offset, ap=[[0, nparts]] + src_ap.ap
    )


def _bcast_parts(src_ap, nparts):
    """Partition-stride-0 view of a single-partition SBUF AP (broadcast read)."""
    return bass.AP(
        tensor=src_ap.tensor, offset=src_ap.offset, ap=[[0, nparts]] + src_ap.ap[1:]
    )


def _emit(tc, io):
    nc = tc.nc
    ctx_lp = nc.allow_low_precision(
        reason="bf16 matmul path validated against 2e-2 harness tolerance"
    )
    ctx_lp.__enter__()

    # ---- long-lived pools (SBUF stack is LIFO; order matters) ----
    small = tc.alloc_tile_pool(name="small", bufs=1)
    utp = tc.alloc_tile_pool(name="utp", bufs=1)
    ktp = tc.alloc_tile_pool(name="ktp", bufs=1)
    qtp = tc.alloc_tile_pool(name="qtp", bufs=1)
    vp = tc.alloc_tile_pool(name="vp", bufs=1)
    wz = tc.alloc_tile_pool(name="wz", bufs=1)
    stat = tc.alloc_tile_pool(name="stat", bufs=1)

    bqk = small.tile([128, 16], F32)
    eps_t = small.tile([128, 1], F32)
    cgb = small.tile([128, 2, E], BF16)  # gamma, beta (bf16: 2x DVE mode)
    cv_sb = small.tile([1, E], BF16)  # cvec as a single-row matmul operand
    ones_sb = small.tile([1, 128], BF16)
    eye_sb = small.tile([128, 128], BF16)  # identity for PE transposes

    UTp = utp.tile([128, H // 2, QSL], BF16)  # normalized per-head-pair O^T
    KT = ktp.tile([128, 8, N2], BF16)
    QT = qtp.tile([128, 8, QSL], BF16)
    Vp = vp.tile([128, 16, H, D + 1], BF16)
    # ones column: contiguous DMA into a staging tile, then one strided DVE
    # copy (a direct DMA would shatter into 32k 2-byte descriptors)
    ones_st = small.tile([128, 256], BF16)
    nc.sync.dma_start(out=ones_st, in_=_bcast_rows(io["ones_bf"][:]))
    nc.vector.tensor_copy(
        out=Vp[:, :, :, D : D + 1],
        in_=ones_st.rearrange("p (a b c) -> p a b c", b=H, c=1),
    )

    wo_sb = wz.tile([128, 8, E], BF16)
    Z = wz.tile([128, 4, E], BF16)

    kt_sb = stat.tile([128, 4, N2], BF16)
    vt_sb = stat.tile([128, 4, N2], BF16)
    wk_sb = stat.tile([128, 4, E], BF16)
    wv_sb = stat.tile([128, 4, E], BF16)

    # staging for Q projection only (released before attention)
    wqq = tc.alloc_tile_pool(name="wqq", bufs=1)
    wq_sb = wqq.tile([128, 4, E], BF16)
    qt_sb = wqq.tile([128, 4, QSL], BF16)

    # ---- input DMAs in consumption order (per-cc splits let the first
    # matmuls start as soon as their own slice lands) ----
    for cc in range(4):
        nc.sync.dma_start(out=qt_sb[:, cc, :], in_=io["qT"][128 * cc : 128 * (cc + 1), :])
        nc.sync.dma_start(
            out=wq_sb[:, cc, :], in_=io["Wq"][128 * cc : 128 * (cc + 1), :]
        )
    nc.sync.dma_start(out=bqk[:, 0:8], in_=io["bq"][:].rearrange("(ec p) -> p ec", p=128))
    nc.vector.memset(eps_t, EPS)
    nc.sync.dma_start(out=wk_sb, in_=io["Wk"][:, :].rearrange("(cc p) e -> p cc e", p=128))
    nc.sync.dma_start(out=bqk[:, 8:16], in_=io["bk"][:].rearrange("(ec p) -> p ec", p=128))
    for ks in range(4):
        nc.sync.dma_start(
            out=kt_sb[:, :, 512 * ks : 512 * (ks + 1)],
            in_=io["kT"][:, 512 * ks : 512 * (ks + 1)].rearrange(
                "(cc p) k -> p cc k", p=128
            ),
        )
    nc.sync.dma_start(out=wv_sb, in_=io["Wv"][:, :].rearrange("(cc p) e -> p cc e", p=128))
    for ks in range(4):
        nc.sync.dma_start(
            out=vt_sb[:, :, 512 * ks : 512 * (ks + 1)],
            in_=io["vT"][:, 512 * ks : 512 * (ks + 1)].rearrange(
                "(cc p) k -> p cc k", p=128
            ),
        )
    nc.sync.dma_start(out=wo_sb, in_=io["Wo"][:, :].rearrange("(pp p) n -> p pp n", p=128))
    nc.sync.dma_start(out=cgb[:, 0, :], in_=_bcast_rows(io["gamma"][:]))
    nc.sync.dma_start(out=cgb[:, 1, :], in_=_bcast_rows(io["beta"][:]))
    nc.sync.dma_start(out=cv_sb[0:1, :], in_=io["cvec"][:])
    nc.sync.dma_start(out=ones_sb, in_=io["ones_bf"][0:128])
    nc.sync.dma_start(out=eye_sb, in_=io["eye_bf"][:, :])

    ps_kv = tc.alloc_tile_pool(name="ps_kv", bufs=2, space="PSUM")
    ps_st = tc.alloc_tile_pool(name="ps_st", bufs=2, space="PSUM")
    ps_ut = tc.alloc_tile_pool(name="ps_ut", bufs=1, space="PSUM")
    ps_tr = tc.alloc_tile_pool(name="ps_tr", bufs=1, space="PSUM")

    # ---- Q projection (all 8 e-chunks) ----
    for ec in range(8):
        ps = ps_kv.tile([128, QSL], F32, tag="ps_kv", name=f"q_{ec}")
        for cc in range(4):
            nc.tensor.matmul(
                ps,
                wq_sb[:, cc, 128 * ec : 128 * (ec + 1)],
                qt_sb[:, cc, :],
                start=(cc == 0),
                stop=(cc == 3),
            )
        nc.vector.tensor_scalar_add(QT[:, ec, :], ps, bqk[:, ec : ec + 1])
    wqq.release()

    # ---- interleavable projection work units ----
    def k_unit(ec, ks):
        ps = ps_kv.tile([128, 512], F32, tag="ps_kv", name=f"k_{ec}_{ks}")
        for cc in range(4):
            nc.tensor.matmul(
                ps,
                wk_sb[:, cc, 128 * ec : 128 * (ec + 1)],
                kt_sb[:, cc, 512 * ks : 512 * (ks + 1)],
                start=(cc == 0),
                stop=(cc == 3),
            )
        nc.vector.tensor_scalar_add(
            KT[:, ec, 512 * ks : 512 * (ks + 1)], ps, bqk[:, 8 + ec : 9 + ec]
        )

    def v_unit(ii, ds):
        ps = ps_kv.tile([128, 512], F32, tag="ps_kv", name=f"v_{ii}_{ds}")
        for cc in range(4):
            nc.tensor.matmul(
                ps,
                vt_sb[:, cc, 128 * ii : 128 * (ii + 1)],
                wv_sb[:, cc, 512 * ds : 512 * (ds + 1)],
                start=(cc == 0),
                stop=(cc == 3),
            )
        nc.vector.tensor_copy(
            out=Vp[:, ii, 8 * ds : 8 * (ds + 1), 0:D],
            in_=ps.rearrange("p (h d) -> p h d", d=D),
        )

    # K head-pair 0: first two k-slices up front, the last two ride in pair
    # 0's first score steps (their kt DMA slices land late; pair-0 scores only
    # consume them from step 8 on).
    for ks in range(2):
        k_unit(0, ks)

    # ---- attention ----
    watn = tc.alloc_tile_pool(name="watn", bufs=1, side="right")

    # Deferred PE transposes / pair copies: flushed after the NEXT head's
    # score block, when the DVE normalizes they depend on have had a full
    # head window (~8us) to complete, so the PE never waits on them.
    pending = []

    def flush_pending():
        for f in pending:
            f()
        pending.clear()

    for hc in range(8):
        # V units ride in the FIRST head's score steps (its ut matmuls consume
        # them at the end of those steps); K units for the next pair can sit
        # anywhere in this pair, so they ride in the second head's steps.
        units_a, units_b = [], []
        if hc == 0:
            units_a += [(lambda ks=ks: k_unit(0, ks)) for ks in (2, 3)]
            units_a += [(lambda ii=ii: v_unit(ii, 0)) for ii in range(16)]
        if hc == 4:
            units_a += [(lambda ii=ii: v_unit(ii, 1)) for ii in range(16)]
        if hc < 7:
            units_b += [(lambda ks=ks: k_unit(hc + 1, ks)) for ks in range(4)]
        slots = [[] for _ in range(16)]
        for idx, u in enumerate(units_a):
            slots[(idx * 8) // len(units_a)].append(u)
        for idx, u in enumerate(units_b):
            slots[8 + (idx * 8) // len(units_b)].append(u)

        def head_block(hh, h):
            # Scores/exp steps with the O' accumulation interleaved one step
            # behind (ut chunk i emitted right after exp i//2), so the ACT exp
            # stream never sees an inter-head batch bubble.
            #
            # O'[q-chunk, d|denominator]: lhsT is the E chunk, rhs is V' —
            # bills 65 rows/matmul instead of 512 (the model charges by output
            # free size, and d+1=65 is the small dim). The denominator lands
            # as a COLUMN [128,1]: per-partition, so normalization is a plain
            # tensor_scalar, no broadcast needed. One zero-region (2KB bank)
            # holds all four q-chunk groups: the first matmul's start marks
            # the whole bank pending-zero (first write per byte overwrites),
            # the very last matmul stops it.
            utn = ps_ut.tile([128, 4, 128], F32, tag="utn", bufs=1, name=f"utn_{h}")
            ets = []

            def ut_chunks(lo, hi):
                for i in range(lo, hi):
                    for qc in range(4):
                        nc.tensor.matmul(
                            utn[:, qc, 0 : D + 1],
                            ets[i // 2][:, i % 2, 128 * qc : 128 * (qc + 1)],
                            Vp[:, i, h, :],
                            start=(i == 0 and qc == 0),
                            stop=(i == 15 and qc == 3),
                        )

            pb = 64 * hh
            for j in range(8):
                st = ps_st.tile([128, 2, 512], F32, tag="st", name=f"st_{h}_{j}")
                for jj in range(2):
                    i = 2 * j + jj
                    nc.tensor.matmul(
                        st[:, jj, :],
                        KT[pb : pb + 64, hc, 128 * i : 128 * (i + 1)],
                        QT[pb : pb + 64, hc, :],
                        start=True,
                        stop=True,
                    )
                for u in slots[8 * hh + j]:
                    u()
                et = watn.tile(
                    [128, 2, 512], BF16, tag="et", bufs=8, name=f"et_{h}_{j}"
                )
                nc.scalar.activation(out=et, in_=st, func=AF.Exp, scale=SCALE)
                ets.append(et)
                if j == 2:
                    # previous head's transposes (their DVE normalizes have
                    # had two score steps to drain)
                    flush_pending()
                if j >= 1:
                    ut_chunks(2 * (j - 1), 2 * j)
            ut_chunks(14, 16)
            On = watn.tile([128, 4, D], BF16, tag="on", bufs=2, name=f"on_{h}")
            for qc in range(4):
                rcp = watn.tile([128, 1], F32, tag="rcp", bufs=8, name=f"rcp_{h}_{qc}")
                nc.vector.reciprocal(out=rcp, in_=utn[:, qc, D : D + 1])
                nc.vector.tensor_scalar_mul(On[:, qc, :], utn[:, qc, 0:D], rcp)
            return On

        tr = ps_tr.tile([128, QSL], BF16, tag="tr", bufs=1, name=f"tr_{hc}")

        def tr_flush(hh, On, tr):
            # PE transposes restore the [d, q] layout the Z matmul needs;
            # head parity picks the output partition half (no DMA shift).
            for qc in range(4):
                nc.tensor.matmul(
                    tr[64 * hh : 64 * hh + 64, 128 * qc : 128 * (qc + 1)],
                    On[:, qc, :],
                    eye_sb,
                    is_transpose=True,
                )

        for hh in range(2):
            h = 2 * hc + hh
            On = head_block(hh, h)
            pending.append(lambda hh=hh, On=On, tr=tr: tr_flush(hh, On, tr))
        pending.append(
            lambda hc=hc, tr=tr: nc.vector.tensor_copy(out=UTp[:, hc, :], in_=tr)
        )

    flush_pending()
    ps_tr.release()
    ps_ut.release()
    ps_st.release()

    # ---- output projection + LayerNorm ----
    zw = tc.alloc_tile_pool(name="zw", bufs=2)
    ps_z = tc.alloc_tile_pool(name="ps_z", bufs=4, space="PSUM")

    for qc in range(4):
        stats = zw.tile([128, 2, 6], F32, tag="stats")
        for ns in range(2):
            ps = ps_z.tile([128, 512], F32, tag="ps_z", name=f"z_{qc}_{ns}")
            for p8 in range(8):
                nc.tensor.matmul(
                    ps,
                    UTp[:, p8, 128 * qc : 128 * (qc + 1)],
                    wo_sb[:, p8, 512 * ns : 512 * (ns + 1)],
                    start=(p8 == 0),
                    stop=False,
                )
            # + bv@Wo + bo folded in as a rank-1 update (ones ⊗ cvec)
            nc.tensor.matmul(
                ps,
                ones_sb,
                cv_sb[0:1, 512 * ns : 512 * (ns + 1)],
                start=False,
                stop=True,
            )
            # stats straight off PSUM (cvec already folded in), in parallel
            # with the PSUM -> SBUF staging on the otherwise-idle ACT engine
            nc.vector.bn_stats(out=stats[:, ns, :], in_=ps)
            nc.scalar.copy(out=Z[:, qc, 512 * ns : 512 * (ns + 1)], in_=ps)
        x = Z[:, qc, :]
        mv = zw.tile([128, 2], F32, tag="mv")
        nc.vector.bn_aggr(out=mv, in_=stats)
        std = zw.tile([128, 1], F32, tag="std")
        nc.scalar.activation(out=std, in_=mv[:, 1:2], func=AF.Sqrt, bias=eps_t)
        rstd = zw.tile([128, 1], F32, tag="rstd")
        nc.vector.reciprocal(out=rstd, in_=std)
        y = zw.tile([128, E], BF16, tag="y")
        yg = zw.tile([128, E], BF16, tag="yg")
        yf = zw.tile([128, E], F32, tag="yf")
        for ns in range(2):
            sl = slice(512 * ns, 512 * (ns + 1))
            nc.vector.tensor_scalar(
                out=y[:, sl],
                in0=x[:, sl],
                scalar1=mv[:, 0:1],
                scalar2=rstd,
                op0=OP.subtract,
                op1=OP.mult,
            )
            nc.vector.tensor_mul(yg[:, sl], y[:, sl], cgb[:, 0, sl])
            if qc < 3:
                nc.gpsimd.tensor_add(yf[:, sl], yg[:, sl], cgb[:, 1, sl])
            else:
                nc.vector.tensor_add(yf[:, sl], yg[:, sl], cgb[:, 1, sl])
            nc.sync.dma_start(
                out=io["out"][128 * qc : 128 * (qc + 1), sl], in_=yf[:, sl]
            )

    ps_z.release()
    zw.release()
    watn.release()
    ps_kv.release()
    stat.release()
    wz.release()
    vp.release()
    qtp.release()
    ktp.release()
    utp.release()
    small.release()
    ctx_lp.__exit__(None, None, None)


def build_bass():
    nc = bacc.Bacc("TRN2", target_bir_lowering=False, debug=False)
    io = {}
    io["qT"] = nc.declare_dram_parameter("qT", [CQ, QSL], BF16, isOutput=False)
    io["kT"] = nc.declare_dram_parameter("kT", [CKV, N2], BF16, isOutput=False)
    io["vT"] = nc.declare_dram_parameter("vT", [CKV, N2], BF16, isOutput=False)
    io["Wq"] = nc.declare_dram_parameter("Wq", [CQ, E], BF16, isOutput=False)
    io["Wk"] = nc.declare_dram_parameter("Wk", [CKV, E], BF16, isOutput=False)
    io["Wv"] = nc.declare_dram_parameter("Wv", [CKV, E], BF16, isOutput=False)
    io["Wo"] = nc.declare_dram_parameter("Wo", [E, E], BF16, isOutput=False)
    io["bq"] = nc.declare_dram_parameter("bq", [E], F32, isOutput=False)
    io["bk"] = nc.declare_dram_parameter("bk", [E], F32, isOutput=False)
    io["ones_bf"] = nc.declare_dram_parameter("ones_bf", [256], BF16, isOutput=False)
    io["eye_bf"] = nc.declare_dram_parameter("eye_bf", [128, 128], BF16, isOutput=False)
    io["cvec"] = nc.declare_dram_parameter("cvec", [E], BF16, isOutput=False)
    io["gamma"] = nc.declare_dram_parameter("gamma", [E], BF16, isOutput=False)
    io["beta"] = nc.declare_dram_parameter("beta", [E], BF16, isOutput=False)
    io["out"] = nc.declare_dram_parameter("out", [QSL, E], F32, isOutput=True)

    with tile.TileContext(nc) as tc:
        _emit(tc, io)
    nc.compile()
    return nc


_NC_CACHE = None


def _get_nc():
    global _NC_CACHE
    if _NC_CACHE is None:
        _NC_CACHE = build_bass()
    return _NC_CACHE


def make_in_maps(query, key, value, Wq, bq, Wk, bk, Wv, bv, Wo, bo, gamma, beta):
    f = np.float32
    cvec = (bv.astype(f) @ Wo.astype(f) + bo.astype(f)).astype(f)
    shared = {
        "Wq": np.ascontiguousarray(Wq).astype(NPBF16),
        "Wk": np.ascontiguousarray(Wk).astype(NPBF16),
        "Wv": np.ascontiguousarray(Wv).astype(NPBF16),
        "Wo": np.ascontiguousarray(Wo).astype(NPBF16),
        "bq": np.ascontiguousarray(bq, f),
        "bk": np.ascontiguousarray(bk, f),
        "cvec": cvec.astype(NPBF16),
        "ones_bf": np.ones(256, NPBF16),
        "eye_bf": np.eye(128, dtype=f).astype(NPBF16),
        "gamma": np.ascontiguousarray(gamma).astype(NPBF16),
        "beta": np.ascontiguousarray(beta).astype(NPBF16),
    }
    kT = [np.ascontiguousarray(key[b].T).astype(NPBF16) for b in range(B)]
    vT = [np.ascontiguousarray(value[b].T).astype(NPBF16) for b in range(B)]
    in_maps = []
    for c in range(NCORES):
        b, r = divmod(c, NCORES // B)
        qT = np.ascontiguousarray(query[b, QSL * r : QSL * (r + 1), :].T).astype(NPBF16)
        m = dict(shared)
        m.update({"qT": qT, "kT": kT[b], "vT": vT[b]})
        in_maps.append(m)
    return in_maps


def kernel(**inputs):
    nc = _get_nc()
    in_maps = make_in_maps(**inputs)
    res = run_bass_kernel_spmd(nc, in_maps, core_ids=list(range(NCORES)))
    out = np.empty((B, N1, E), np.float32)
    for c in range(NCORES):
        b, r = divmod(c, NCORES // B)
        out[b, QSL * r : QSL * (r + 1), :] = res.results[c]["out"]
    return out


# revision 82
# speedup vs baseline: 1.0092x; 1.0092x over previous
"""Cross-attention layer (B=2, N1=N2=2048, 16 heads x 64, fp32) on 8 trn2 cores.

Sharding: core c = (batch b=c//4, query-row slice r=c%4 of 512 rows).
K/V projections are replicated within a batch group (no collectives needed);
every other stage is fully data-parallel.  Output is a pure host-side concat.

Schedule (cost-model driven): the exp stream on ACT and the matmul stream on
PE are the two big engine budgets, so the K projection for head-pairs 1..7 and
the whole V projection are interleaved INTO the attention loop where the PE
would otherwise wait on ACT.  All matmul inputs are bf16 (host pre-casts;
matmul billing is identical to fp32r but DMA/SBUF halve), accumulation stays
fp32 in PSUM, LayerNorm runs fp32 stats over a bf16 staging of Z.

Device algorithm per core:
  QT[e,q] = (Wq^T q^T + bq)          bf16, e on partitions
  KT[e,k] = (Wk^T k^T + bk)          bf16 (head-pair ec emitted just-in-time)
  V'[k,(h,d|1)] = v^T^T Wv, ones column per head (softmax denominator trick)
  per head h:  ST[k,q] = KT_h^T QT_h         (16 k-chunks, 2 chunks/PSUM tile)
               ET = exp(ST/8)                 (ACT, 1024-wide reads)
               UT'[65,q] += V'_h^T ET         (PSUM accumulate)
      row 64 of UT' is the softmax denominator s[q]; 1/s broadcast across
      partitions with a stride-0 SBUF->SBUF DMA, then one DVE multiply.
  Z[q,n] = sum over head-pairs UTpair^T Wo-rows + (bv@Wo + bo)
  out    = LayerNorm(Z) * gamma + beta
"""

import sys

for _p in ("/opt/trn_rl_repo",):
    if _p not in sys.path:
        sys.path.insert(0, _p)

import ml_dtypes
import numpy as np

import concourse.bass as bass
import concourse.mybir as mybir
import concourse.tile as tile
from concourse import bacc
from concourse.bass_utils import run_bass_kernel_spmd

F32 = mybir.dt.float32
F32R = mybir.dt.float32r
BF16 = mybir.dt.bfloat16
AF = mybir.ActivationFunctionType
OP = mybir.AluOpType
NPBF16 = np.dtype(ml_dtypes.bfloat16)

B = 2
N1 = 2048
N2 = 2048
CQ = 512  # query feature dim
CKV = 512  # key/value feature dim
E = 1024  # embed dim
H = 16  # heads
D = 64  # head dim
EPS = 1e-5
NCORES = 8
QSL = N1 * B // NCORES  # 512 query rows per core
SCALE = float(D) ** -0.5


def _bcast_rows(src_ap, nparts=128):
    """Broadcast a 1-D DRAM AP across partitions (step-0 partition dim)."""
    return bass.AP(
        tensor=src_ap.tensor, offset=src_ap.offset, ap=[[0, nparts]] + src_ap.ap
    )


def _bcast_parts(src_ap, nparts):
    """Partition-stride-0 view of a single-partition SBUF AP (broadcast read)."""
    return bass.AP(
        tensor=src_ap.tensor, offset=src_ap.offset, ap=[[0, nparts]] + src_ap.ap[1:]
    )


def _emit(tc, io):
    nc = tc.nc
    ctx_lp = nc.allow_low_precision(
        reason="bf16 matmul path validated against 2e-2 harness tolerance"
    )
    ctx_lp.__enter__()

    # ---- long-lived pools (SBUF stack is LIFO; order matters) ----
    small = tc.alloc_tile_pool(name="small", bufs=1)
    utp = tc.alloc_tile_pool(name="utp", bufs=1)
    ktp = tc.alloc_tile_pool(name="ktp", bufs=1)
    qtp = tc.alloc_tile_pool(name="qtp", bufs=1)
    vp = tc.alloc_tile_pool(name="vp", bufs=1)
    wz = tc.alloc_tile_pool(name="wz", bufs=1)
    stat = tc.alloc_tile_pool(name="stat", bufs=1)

    bqk = small.tile([128, 16], F32)
    eps_t = small.tile([128, 1], F32)
    cgb = small.tile([128, 2, E], BF16)  # gamma, beta (bf16: 2x DVE mode)
    cv_sb = small.tile([1, E], BF16)  # cvec as a single-row matmul operand
    ones_sb = small.tile([1, 128], BF16)
    eye_sb = small.tile([128, 128], BF16)  # identity for PE transposes

    UTp = utp.tile([128, H // 2, QSL], BF16)  # normalized per-head-pair O^T
    KT = ktp.tile([128, 8, N2], BF16)
    QT = qtp.tile([128, 8, QSL], BF16)
    Vp = vp.tile([128, 16, H, D + 1], BF16)
    # ones column: contiguous DMA into a staging tile, then one strided DVE
    # copy (a direct DMA would shatter into 32k 2-byte descriptors)
    ones_st = small.tile([128, 256], BF16)
    nc.sync.dma_start(out=ones_st, in_=_bcast_rows(io["ones_bf"][:]))
    nc.vector.tensor_copy(
        out=Vp[:, :, :, D : D + 1],
        in_=ones_st.rearrange("p (a b c) -> p a b c", b=H, c=1),
    )

    wo_sb = wz.tile([128, 8, E], BF16)
    Z = wz.tile([128, 4, E], BF16)

    kt_sb = stat.tile([128, 4, N2], BF16)
    vt_sb = stat.tile([128, 4, N2], BF16)
    wk_sb = stat.tile([128, 4, E], BF16)
    wv_sb = stat.tile([128, 4, E], BF16)

    # staging for Q projection only (released before attention)
    wqq = tc.alloc_tile_pool(name="wqq", bufs=1)
    wq_sb = wqq.tile([128, 4, E], BF16)
    qt_sb = wqq.tile([128, 4, QSL], BF16)

    # ---- input DMAs in consumption order (per-cc splits let the first
    # matmuls start as soon as their own slice lands) ----
    for cc in range(4):
        nc.sync.dma_start(out=qt_sb[:, cc, :], in_=io["qT"][128 * cc : 128 * (cc + 1), :])
        nc.sync.dma_start(
            out=wq_sb[:, cc, :], in_=io["Wq"][128 * cc : 128 * (cc + 1), :]
        )
    nc.sync.dma_start(out=bqk[:, 0:8], in_=io["bq"][:].rearrange("(ec p) -> p ec", p=128))
    nc.vector.memset(eps_t, EPS)
    nc.sync.dma_start(out=wk_sb, in_=io["Wk"][:, :].rearrange("(cc p) e -> p cc e", p=128))
    nc.sync.dma_start(out=bqk[:, 8:16], in_=io["bk"][:].rearrange("(ec p) -> p ec", p=128))
    for ks in range(4):
        nc.sync.dma_start(
            out=kt_sb[:, :, 512 * ks : 512 * (ks + 1)],
            in_=io["kT"][:, 512 * ks : 512 * (ks + 1)].rearrange(
                "(cc p) k -> p cc k", p=128
            ),
        )
    nc.sync.dma_start(out=wv_sb, in_=io["Wv"][:, :].rearrange("(cc p) e -> p cc e", p=128))
    for ks in range(4):
        nc.sync.dma_start(
            out=vt_sb[:, :, 512 * ks : 512 * (ks + 1)],
            in_=io["vT"][:, 512 * ks : 512 * (ks + 1)].rearrange(
                "(cc p) k -> p cc k", p=128
            ),
        )
    nc.sync.dma_start(out=wo_sb, in_=io["Wo"][:, :].rearrange("(pp p) n -> p pp n", p=128))
    nc.sync.dma_start(out=cgb[:, 0, :], in_=_bcast_rows(io["gamma"][:]))
    nc.sync.dma_start(out=cgb[:, 1, :], in_=_bcast_rows(io["beta"][:]))
    nc.sync.dma_start(out=cv_sb[0:1, :], in_=io["cvec"][:])
    nc.sync.dma_start(out=ones_sb, in_=io["ones_bf"][0:128])
    nc.sync.dma_start(out=eye_sb, in_=io["eye_bf"][:, :])

    ps_kv = tc.alloc_tile_pool(name="ps_kv", bufs=2, space="PSUM")
    ps_st = tc.alloc_tile_pool(name="ps_st", bufs=2, space="PSUM")
    ps_ut = tc.alloc_tile_pool(name="ps_ut", bufs=1, space="PSUM")
    ps_tr = tc.alloc_tile_pool(name="ps_tr", bufs=1, space="PSUM")

    # ---- Q projection (all 8 e-chunks) ----
    for ec in range(8):
        ps = ps_kv.tile([128, QSL], F32, tag="ps_kv", name=f"q_{ec}")
        for cc in range(4):
            nc.tensor.matmul(
                ps,
                wq_sb[:, cc, 128 * ec : 128 * (ec + 1)],
                qt_sb[:, cc, :],
                start=(cc == 0),
                stop=(cc == 3),
            )
        nc.vector.tensor_scalar_add(QT[:, ec, :], ps, bqk[:, ec : ec + 1])
    wqq.release()

    # ---- interleavable projection work units ----
    def k_unit(ec, ks):
        ps = ps_kv.tile([128, 512], F32, tag="ps_kv", name=f"k_{ec}_{ks}")
        for cc in range(4):
            nc.tensor.matmul(
                ps,
                wk_sb[:, cc, 128 * ec : 128 * (ec + 1)],
                kt_sb[:, cc, 512 * ks : 512 * (ks + 1)],
                start=(cc == 0),
                stop=(cc == 3),
            )
        nc.vector.tensor_scalar_add(
            KT[:, ec, 512 * ks : 512 * (ks + 1)], ps, bqk[:, 8 + ec : 9 + ec]
        )

    def v_unit(ii, ds):
        ps = ps_kv.tile([128, 512], F32, tag="ps_kv", name=f"v_{ii}_{ds}")
        for cc in range(4):
            nc.tensor.matmul(
                ps,
                vt_sb[:, cc, 128 * ii : 128 * (ii + 1)],
                wv_sb[:, cc, 512 * ds : 512 * (ds + 1)],
                start=(cc == 0),
                stop=(cc == 3),
            )
        nc.vector.tensor_copy(
            out=Vp[:, ii, 8 * ds : 8 * (ds + 1), 0:D],
            in_=ps.rearrange("p (h d) -> p h d", d=D),
        )

    # K head-pair 0: first two k-slices up front, the last two ride in pair
    # 0's first score steps (their kt DMA slices land late; pair-0 scores only
    # consume them from step 8 on).
    for ks in range(2):
        k_unit(0, ks)

    # ---- attention ----
    watn = tc.alloc_tile_pool(name="watn", bufs=1, side="right")

    # Deferred PE transposes / pair copies: flushed after the NEXT head's
    # score block, when the DVE normalizes they depend on have had a full
    # head window (~8us) to complete, so the PE never waits on them.
    pending = []

    def flush_pending():
        for f in pending:
            f()
        pending.clear()

    for hc in range(8):
        # V units ride in the FIRST head's score steps (its ut matmuls consume
        # them at the end of those steps); K units for the next pair can sit
        # anywhere in this pair, so they ride in the second head's steps.
        units_a, units_b = [], []
        if hc == 0:
            units_a += [(lambda ks=ks: k_unit(0, ks)) for ks in (2, 3)]
            units_a += [(lambda ii=ii: v_unit(ii, 0)) for ii in range(16)]
        if hc == 4:
            units_a += [(lambda ii=ii: v_unit(ii, 1)) for ii in range(16)]
        if hc < 7:
            units_b += [(lambda ks=ks: k_unit(hc + 1, ks)) for ks in range(4)]
        slots = [[] for _ in range(16)]
        for idx, u in enumerate(units_a):
            slots[(idx * 8) // len(units_a)].append(u)
        for idx, u in enumerate(units_b):
            slots[8 + (idx * 8) // len(units_b)].append(u)

        def head_block(hh, h):
            # Scores/exp steps with the O' accumulation interleaved one step
            # behind (ut chunk i emitted right after exp i//2), so the ACT exp
            # stream never sees an inter-head batch bubble.
            #
            # O'[q-chunk, d|denominator]: lhsT is the E chunk, rhs is V' —
            # bills 65 rows/matmul instead of 512 (the model charges by output
            # free size, and d+1=65 is the small dim). The denominator lands
            # as a COLUMN [128,1]: per-partition, so normalization is a plain
            # tensor_scalar, no broadcast needed. One zero-region (2KB bank)
            # holds all four q-chunk groups: the first matmul's start marks
            # the whole bank pending-zero (first write per byte overwrites),
            # the very last matmul stops it.
            utn = ps_ut.tile([128, 4, 128], F32, tag="utn", bufs=1, name=f"utn_{h}")
            ets = []

            def ut_chunks(lo, hi):
                for i in range(lo, hi):
                    for qc in range(4):
                        nc.tensor.matmul(
                            utn[:, qc, 0 : D + 1],
                            ets[i // 2][:, i % 2, 128 * qc : 128 * (qc + 1)],
                            Vp[:, i, h, :],
                            start=(i == 0 and qc == 0),
                            stop=(i == 15 and qc == 3),
                        )

            pb = 64 * hh
            for j in range(8):
                st = ps_st.tile([128, 2, 512], F32, tag="st", name=f"st_{h}_{j}")
                for jj in range(2):
                    i = 2 * j + jj
                    nc.tensor.matmul(
                        st[:, jj, :],
                        KT[pb : pb + 64, hc, 128 * i : 128 * (i + 1)],
                        QT[pb : pb + 64, hc, :],
                        start=True,
                        stop=True,
                    )
                for u in slots[8 * hh + j]:
                    u()
                et = watn.tile(
                    [128, 2, 512], BF16, tag="et", bufs=8, name=f"et_{h}_{j}"
                )
                nc.scalar.activation(out=et, in_=st, func=AF.Exp, scale=SCALE)
                ets.append(et)
                if j == 2:
                    # previous head's transposes (their DVE normalizes have
                    # had two score steps to drain)
                    flush_pending()
            ut_chunks(0, 16)
            On = watn.tile([128, 4, D], BF16, tag="on", bufs=2, name=f"on_{h}")
            for qc in range(4):
                rcp = watn.tile([128, 1], F32, tag="rcp", bufs=8, name=f"rcp_{h}_{qc}")
                nc.vector.reciprocal(out=rcp, in_=utn[:, qc, D : D + 1])
                nc.vector.tensor_scalar_mul(On[:, qc, :], utn[:, qc, 0:D], rcp)
            return On

        tr = ps_tr.tile([128, QSL], BF16, tag="tr", bufs=1, name=f"tr_{hc}")

        def tr_flush(hh, On, tr):
            # PE transposes restore the [d, q] layout the Z matmul needs;
            # head parity picks the output partition half (no DMA shift).
            for qc in range(4):
                nc.tensor.matmul(
                    tr[64 * hh : 64 * hh + 64, 128 * qc : 128 * (qc + 1)],
                    On[:, qc, :],
                    eye_sb,
                    is_transpose=True,
                )

        for hh in range(2):
            h = 2 * hc + hh
            On = head_block(hh, h)
            pending.append(lambda hh=hh, On=On, tr=tr: tr_flush(hh, On, tr))
        pending.append(
            lambda hc=hc, tr=tr: nc.vector.tensor_copy(out=UTp[:, hc, :], in_=tr)
        )

    flush_pending()
    ps_tr.release()
    ps_ut.release()
    ps_st.release()

    # ---- output projection + LayerNorm ----
    zw = tc.alloc_tile_pool(name="zw", bufs=2)
    ps_z = tc.alloc_tile_pool(name="ps_z", bufs=4, space="PSUM")

    for qc in range(4):
        stats = zw.tile([128, 2, 6], F32, tag="stats")
        for ns in range(2):
            ps = ps_z.tile([128, 512], F32, tag="ps_z", name=f"z_{qc}_{ns}")
            for p8 in range(8):
                nc.tensor.matmul(
                    ps,
                    UTp[:, p8, 128 * qc : 128 * (qc + 1)],
                    wo_sb[:, p8, 512 * ns : 512 * (ns + 1)],
                    start=(p8 == 0),
                    stop=False,
                )
            # + bv@Wo + bo folded in as a rank-1 update (ones ⊗ cvec)
            nc.tensor.matmul(
                ps,
                ones_sb,
                cv_sb[0:1, 512 * ns : 512 * (ns + 1)],
                start=False,
                stop=True,
            )
            # stats straight off PSUM (cvec already folded in), in parallel
            # with the PSUM -> SBUF staging on the otherwise-idle ACT engine
            nc.vector.bn_stats(out=stats[:, ns, :], in_=ps)
            nc.scalar.copy(out=Z[:, qc, 512 * ns : 512 * (ns + 1)], in_=ps)
        x = Z[:, qc, :]
        mv = zw.tile([128, 2], F32, tag="mv")
        nc.vector.bn_aggr(out=mv, in_=stats)
        std = zw.tile([128, 1], F32, tag="std")
        nc.scalar.activation(out=std, in_=mv[:, 1:2], func=AF.Sqrt, bias=eps_t)
        rstd = zw.tile([128, 1], F32, tag="rstd")
        nc.vector.reciprocal(out=rstd, in_=std)
        y = zw.tile([128, E], BF16, tag="y")
        yg = zw.tile([128, E], BF16, tag="yg")
        yf = zw.tile([128, E], F32, tag="yf")
        for ns in range(2):
            sl = slice(512 * ns, 512 * (ns + 1))
            nc.vector.tensor_scalar(
                out=y[:, sl],
                in0=x[:, sl],
                scalar1=mv[:, 0:1],
                scalar2=rstd,
                op0=OP.subtract,
                op1=OP.mult,
            )
            nc.vector.tensor_mul(yg[:, sl], y[:, sl], cgb[:, 0, sl])
            if qc < 3:
                nc.gpsimd.tensor_add(yf[:, sl], yg[:, sl], cgb[:, 1, sl])
            else:
                nc.vector.tensor_add(yf[:, sl], yg[:, sl], cgb[:, 1, sl])
            nc.sync.dma_start(
                out=io["out"][128 * qc : 128 * (qc + 1), sl], in_=yf[:, sl]
            )

    ps_z.release()
    zw.release()
    watn.release()
    ps_kv.release()
    stat.release()
    wz.release()
    vp.release()
    qtp.release()
    ktp.release()
    utp.release()
    small.release()
    ctx_lp.__exit__(None, None, None)


def build_bass():
    nc = bacc.Bacc("TRN2", target_bir_lowering=False, debug=False)
    io = {}
    io["qT"] = nc.declare_dram_parameter("qT", [CQ, QSL], BF16, isOutput=False)
    io["kT"] = nc.declare_dram_parameter("kT", [CKV, N2], BF16, isOutput=False)
    io["vT"] = nc.declare_dram_parameter("vT", [CKV, N2], BF16, isOutput=False)
    io["Wq"] = nc.declare_dram_parameter("Wq", [CQ, E], BF16, isOutput=False)
    io["Wk"] = nc.declare_dram_parameter("Wk", [CKV, E], BF16, isOutput=False)
    io["Wv"] = nc.declare_dram_parameter("Wv", [CKV, E], BF16, isOutput=False)
    io["Wo"] = nc.declare_dram_parameter("Wo", [E, E], BF16, isOutput=False)
    io["bq"] = nc.declare_dram_parameter("bq", [E], F32, isOutput=False)
    io["bk"] = nc.declare_dram_parameter("bk", [E], F32, isOutput=False)
    io["ones_bf"] = nc.declare_dram_parameter("ones_bf", [256], BF16, isOutput=False)
    io["eye_bf"] = nc.declare_dram_parameter("eye_bf", [128, 128], BF16, isOutput=False)
    io["cvec"] = nc.declare_dram_parameter("cvec", [E], BF16, isOutput=False)
    io["gamma"] = nc.declare_dram_parameter("gamma", [E], BF16, isOutput=False)
    io["beta"] = nc.declare_dram_parameter("beta", [E], BF16, isOutput=False)
    io["out"] = nc.declare_dram_parameter("out", [QSL, E], F32, isOutput=True)

    with tile.TileContext(nc) as tc:
        _emit(tc, io)
    nc.compile()
    return nc


_NC_CACHE = None


def _get_nc():
    global _NC_CACHE
    if _NC_CACHE is None:
        _NC_CACHE = build_bass()
    return _NC_CACHE


def make_in_maps(query, key, value, Wq, bq, Wk, bk, Wv, bv, Wo, bo, gamma, beta):
    f = np.float32
    cvec = (bv.astype(f) @ Wo.astype(f) + bo.astype(f)).astype(f)
    shared = {
        "Wq": np.ascontiguousarray(Wq).astype(NPBF16),
        "Wk": np.ascontiguousarray(Wk).astype(NPBF16),
        "Wv": np.ascontiguousarray(Wv).astype(NPBF16),
        "Wo": np.ascontiguousarray(Wo).astype(NPBF16),
        "bq": np.ascontiguousarray(bq, f),
        "bk": np.ascontiguousarray(bk, f),
        "cvec": cvec.astype(NPBF16),
        "ones_bf": np.ones(256, NPBF16),
        "eye_bf": np.eye(128, dtype=f).astype(NPBF16),
        "gamma": np.ascontiguousarray(gamma).astype(NPBF16),
        "beta": np.ascontiguousarray(beta).astype(NPBF16),
    }
    kT = [np.ascontiguousarray(key[b].T).astype(NPBF16) for b in range(B)]
    vT = [np.ascontiguousarray(value[b].T).astype(NPBF16) for b in range(B)]
    in_maps = []
    for c in range(NCORES):
        b, r = divmod(c, NCORES // B)
        qT = np.ascontiguousarray(query[b, QSL * r : QSL * (r + 1), :].T).astype(NPBF16)
        m = dict(shared)
        m.update({"qT": qT, "kT": kT[b], "vT": vT[b]})
        in_maps.append(m)
    return in_maps


def kernel(**inputs):
    nc = _get_nc()
    in_maps = make_in_maps(**inputs)
    res = run_bass_kernel_spmd(nc, in_maps, core_ids=list(range(NCORES)))
    out = np.empty((B, N1, E), np.float32)
    for c in range(NCORES):
        b, r = divmod(c, NCORES // B)
        out[b, QSL * r : QSL * (r + 1), :] = res.results[c]["out"]
    return out


# revision 84
# speedup vs baseline: 1.0104x; 1.0012x over previous
"""Cross-attention layer (B=2, N1=N2=2048, 16 heads x 64, fp32) on 8 trn2 cores.

Sharding: core c = (batch b=c//4, query-row slice r=c%4 of 512 rows).
K/V projections are replicated within a batch group (no collectives needed);
every other stage is fully data-parallel.  Output is a pure host-side concat.

Schedule (cost-model driven): the exp stream on ACT and the matmul stream on
PE are the two big engine budgets, so the K projection for head-pairs 1..7 and
the whole V projection are interleaved INTO the attention loop where the PE
would otherwise wait on ACT.  All matmul inputs are bf16 (host pre-casts;
matmul billing is identical to fp32r but DMA/SBUF halve), accumulation stays
fp32 in PSUM, LayerNorm runs fp32 stats over a bf16 staging of Z.

Device algorithm per core:
  QT[e,q] = (Wq^T q^T + bq)          bf16, e on partitions
  KT[e,k] = (Wk^T k^T + bk)          bf16 (head-pair ec emitted just-in-time)
  V'[k,(h,d|1)] = v^T^T Wv, ones column per head (softmax denominator trick)
  per head h:  ST[k,q] = KT_h^T QT_h         (16 k-chunks, 2 chunks/PSUM tile)
               ET = exp(ST/8)                 (ACT, 1024-wide reads)
               UT'[65,q] += V'_h^T ET         (PSUM accumulate)
      row 64 of UT' is the softmax denominator s[q]; 1/s broadcast across
      partitions with a stride-0 SBUF->SBUF DMA, then one DVE multiply.
  Z[q,n] = sum over head-pairs UTpair^T Wo-rows + (bv@Wo + bo)
  out    = LayerNorm(Z) * gamma + beta
"""

import sys

for _p in ("/opt/trn_rl_repo",):
    if _p not in sys.path:
        sys.path.insert(0, _p)

import ml_dtypes
import numpy as np

import concourse.bass as bass
import concourse.mybir as mybir
import concourse.tile as tile
from concourse import bacc
from concourse.bass_utils import run_bass_kernel_spmd

F32 = mybir.dt.float32
F32R = mybir.dt.float32r
BF16 = mybir.dt.bfloat16
AF = mybir.ActivationFunctionType
OP = mybir.AluOpType
NPBF16 = np.dtype(ml_dtypes.bfloat16)

B = 2
N1 = 2048
N2 = 2048
CQ = 512  # query feature dim
CKV = 512  # key/value feature dim
E = 1024  # embed dim
H = 16  # heads
D = 64  # head dim
EPS = 1e-5
NCORES = 8
QSL = N1 * B // NCORES  # 512 query rows per core
SCALE = float(D) ** -0.5


def _bcast_rows(src_ap, nparts=128):
    """Broadcast a 1-D DRAM AP across partitions (step-0 partition dim)."""
    return bass.AP(
        tensor=src_ap.tensor, offset=src_ap.offset, ap=[[0, nparts]] + src_ap.ap
    )


def _bcast_parts(src_ap, nparts):
    """Partition-stride-0 view of a single-partition SBUF AP (broadcast read)."""
    return bass.AP(
        tensor=src_ap.tensor, offset=src_ap.offset, ap=[[0, nparts]] + src_ap.ap[1:]
    )


def _emit(tc, io):
    nc = tc.nc
    ctx_lp = nc.allow_low_precision(
        reason="bf16 matmul path validated against 2e-2 harness tolerance"
    )
    ctx_lp.__enter__()

    # ---- long-lived pools (SBUF stack is LIFO; order matters) ----
    small = tc.alloc_tile_pool(name="small", bufs=1)
    utp = tc.alloc_tile_pool(name="utp", bufs=1)
    ktp = tc.alloc_tile_pool(name="ktp", bufs=1)
    qtp = tc.alloc_tile_pool(name="qtp", bufs=1)
    vp = tc.alloc_tile_pool(name="vp", bufs=1)
    wz = tc.alloc_tile_pool(name="wz", bufs=1)
    stat = tc.alloc_tile_pool(name="stat", bufs=1)

    bqk = small.tile([128, 16], F32)
    eps_t = small.tile([128, 1], F32)
    cgb = small.tile([128, 2, E], BF16)  # gamma, beta (bf16: 2x DVE mode)
    cv_sb = small.tile([1, E], BF16)  # cvec as a single-row matmul operand
    ones_sb = small.tile([1, 128], BF16)
    eye_sb = small.tile([128, 128], BF16)  # identity for PE transposes

    UTp = utp.tile([128, H // 2, QSL], BF16)  # normalized per-head-pair O^T
    KT = ktp.tile([128, 8, N2], BF16)
    QT = qtp.tile([128, 8, QSL], BF16)
    Vp = vp.tile([128, 16, H, D + 1], BF16)
    # ones column: contiguous DMA into a staging tile, then one strided DVE
    # copy (a direct DMA would shatter into 32k 2-byte descriptors)
    ones_st = small.tile([128, 256], BF16)
    nc.sync.dma_start(out=ones_st, in_=_bcast_rows(io["ones_bf"][:]))
    nc.vector.tensor_copy(
        out=Vp[:, :, :, D : D + 1],
        in_=ones_st.rearrange("p (a b c) -> p a b c", b=H, c=1),
    )

    wo_sb = wz.tile([128, 8, E], BF16)
    Z = wz.tile([128, 4, E], BF16)

    kt_sb = stat.tile([128, 4, N2], BF16)
    vt_sb = stat.tile([128, 4, N2], BF16)
    wk_sb = stat.tile([128, 4, E], BF16)
    wv_sb = stat.tile([128, 4, E], BF16)

    # staging for Q projection only (released before attention)
    wqq = tc.alloc_tile_pool(name="wqq", bufs=1)
    wq_sb = wqq.tile([128, 4, E], BF16)
    qt_sb = wqq.tile([128, 4, QSL], BF16)

    # ---- input DMAs in consumption order (per-cc splits let the first
    # matmuls start as soon as their own slice lands) ----
    for cc in range(4):
        nc.sync.dma_start(out=qt_sb[:, cc, :], in_=io["qT"][128 * cc : 128 * (cc + 1), :])
        nc.sync.dma_start(
            out=wq_sb[:, cc, :], in_=io["Wq"][128 * cc : 128 * (cc + 1), :]
        )
    nc.sync.dma_start(out=bqk[:, 0:8], in_=io["bq"][:].rearrange("(ec p) -> p ec", p=128))
    nc.vector.memset(eps_t, EPS)
    nc.sync.dma_start(out=wk_sb, in_=io["Wk"][:, :].rearrange("(cc p) e -> p cc e", p=128))
    nc.sync.dma_start(out=bqk[:, 8:16], in_=io["bk"][:].rearrange("(ec p) -> p ec", p=128))
    for ks in range(4):
        nc.sync.dma_start(
            out=kt_sb[:, :, 512 * ks : 512 * (ks + 1)],
            in_=io["kT"][:, 512 * ks : 512 * (ks + 1)].rearrange(
                "(cc p) k -> p cc k", p=128
            ),
        )
    nc.sync.dma_start(out=wv_sb, in_=io["Wv"][:, :].rearrange("(cc p) e -> p cc e", p=128))
    for ks in range(4):
        nc.sync.dma_start(
            out=vt_sb[:, :, 512 * ks : 512 * (ks + 1)],
            in_=io["vT"][:, 512 * ks : 512 * (ks + 1)].rearrange(
                "(cc p) k -> p cc k", p=128
            ),
        )
    nc.sync.dma_start(out=wo_sb, in_=io["Wo"][:, :].rearrange("(pp p) n -> p pp n", p=128))
    nc.sync.dma_start(out=cgb[:, 0, :], in_=_bcast_rows(io["gamma"][:]))
    nc.sync.dma_start(out=cgb[:, 1, :], in_=_bcast_rows(io["beta"][:]))
    nc.sync.dma_start(out=cv_sb[0:1, :], in_=io["cvec"][:])
    nc.sync.dma_start(out=ones_sb, in_=io["ones_bf"][0:128])
    nc.sync.dma_start(out=eye_sb, in_=io["eye_bf"][:, :])

    ps_kv = tc.alloc_tile_pool(name="ps_kv", bufs=2, space="PSUM")
    ps_st = tc.alloc_tile_pool(name="ps_st", bufs=2, space="PSUM")
    ps_ut = tc.alloc_tile_pool(name="ps_ut", bufs=1, space="PSUM")
    ps_tr = tc.alloc_tile_pool(name="ps_tr", bufs=1, space="PSUM")

    # ---- Q projection (all 8 e-chunks) ----
    for ec in range(8):
        ps = ps_kv.tile([128, QSL], F32, tag="ps_kv", name=f"q_{ec}")
        for cc in range(4):
            nc.tensor.matmul(
                ps,
                wq_sb[:, cc, 128 * ec : 128 * (ec + 1)],
                qt_sb[:, cc, :],
                start=(cc == 0),
                stop=(cc == 3),
            )
        nc.vector.tensor_scalar_add(QT[:, ec, :], ps, bqk[:, ec : ec + 1])
    wqq.release()

    # ---- interleavable projection work units ----
    def k_unit(ec, ks):
        ps = ps_kv.tile([128, 512], F32, tag="ps_kv", name=f"k_{ec}_{ks}")
        for cc in range(4):
            nc.tensor.matmul(
                ps,
                wk_sb[:, cc, 128 * ec : 128 * (ec + 1)],
                kt_sb[:, cc, 512 * ks : 512 * (ks + 1)],
                start=(cc == 0),
                stop=(cc == 3),
            )
        nc.vector.tensor_scalar_add(
            KT[:, ec, 512 * ks : 512 * (ks + 1)], ps, bqk[:, 8 + ec : 9 + ec]
        )

    def v_unit(ii, ds):
        ps = ps_kv.tile([128, 512], F32, tag="ps_kv", name=f"v_{ii}_{ds}")
        for cc in range(4):
            nc.tensor.matmul(
                ps,
                vt_sb[:, cc, 128 * ii : 128 * (ii + 1)],
                wv_sb[:, cc, 512 * ds : 512 * (ds + 1)],
                start=(cc == 0),
                stop=(cc == 3),
            )
        nc.vector.tensor_copy(
            out=Vp[:, ii, 8 * ds : 8 * (ds + 1), 0:D],
            in_=ps.rearrange("p (h d) -> p h d", d=D),
        )

    # K head-pair 0: first two k-slices up front, the last two ride in pair
    # 0's first score steps (their kt DMA slices land late; pair-0 scores only
    # consume them from step 8 on).
    for ks in range(2):
        k_unit(0, ks)

    # ---- attention ----
    watn = tc.alloc_tile_pool(name="watn", bufs=1, side="right")

    # Deferred PE transposes / pair copies: flushed after the NEXT head's
    # score block, when the DVE normalizes they depend on have had a full
    # head window (~8us) to complete, so the PE never waits on them.
    pending = []

    def flush_pending():
        for f in pending:
            f()
        pending.clear()

    for hc in range(8):
        # V units ride in the FIRST head's score steps (its ut matmuls consume
        # them at the end of those steps); K units for the next pair can sit
        # anywhere in this pair, so they ride in the second head's steps.
        units_a, units_b = [], []
        if hc == 0:
            units_a += [(lambda ks=ks: k_unit(0, ks)) for ks in (2, 3)]
            units_a += [(lambda ii=ii: v_unit(ii, 0)) for ii in range(16)]
        if hc == 4:
            units_a += [(lambda ii=ii: v_unit(ii, 1)) for ii in range(16)]
        if hc < 7:
            units_b += [(lambda ks=ks: k_unit(hc + 1, ks)) for ks in range(4)]
        slots = [[] for _ in range(16)]
        for idx, u in enumerate(units_a):
            slots[(idx * 8) // len(units_a)].append(u)
        for idx, u in enumerate(units_b):
            slots[8 + (idx * 8) // len(units_b)].append(u)

        def head_block(hh, h):
            # Scores/exp steps with the O' accumulation interleaved one step
            # behind (ut chunk i emitted right after exp i//2), so the ACT exp
            # stream never sees an inter-head batch bubble.
            #
            # O'[q-chunk, d|denominator]: lhsT is the E chunk, rhs is V' —
            # bills 65 rows/matmul instead of 512 (the model charges by output
            # free size, and d+1=65 is the small dim). The denominator lands
            # as a COLUMN [128,1]: per-partition, so normalization is a plain
            # tensor_scalar, no broadcast needed. One zero-region (2KB bank)
            # holds all four q-chunk groups: the first matmul's start marks
            # the whole bank pending-zero (first write per byte overwrites),
            # the very last matmul stops it.
            utn = ps_ut.tile([128, 4, 128], F32, tag="utn", bufs=1, name=f"utn_{h}")
            ets = []

            def ut_chunks(lo, hi):
                for i in range(lo, hi):
                    for qc in range(4):
                        nc.tensor.matmul(
                            utn[:, qc, 0 : D + 1],
                            ets[i // 2][:, i % 2, 128 * qc : 128 * (qc + 1)],
                            Vp[:, i, h, :],
                            start=(i == 0 and qc == 0),
                            stop=(i == 15 and qc == 3),
                        )

            pb = 64 * hh
            for j in range(8):
                st = ps_st.tile([128, 2, 512], F32, tag="st", name=f"st_{h}_{j}")
                for jj in range(2):
                    i = 2 * j + jj
                    nc.tensor.matmul(
                        st[:, jj, :],
                        KT[pb : pb + 64, hc, 128 * i : 128 * (i + 1)],
                        QT[pb : pb + 64, hc, :],
                        start=True,
                        stop=True,
                    )
                for u in slots[8 * hh + j]:
                    u()
                et = watn.tile(
                    [128, 2, 512], BF16, tag="et", bufs=8, name=f"et_{h}_{j}"
                )
                nc.scalar.activation(out=et, in_=st, func=AF.Exp, scale=SCALE)
                ets.append(et)

            # previous head's transposes (their DVE normalizes have had a
            # full score block to drain)
            flush_pending()
            ut_chunks(0, 16)
            On = watn.tile([128, 4, D], BF16, tag="on", bufs=2, name=f"on_{h}")
            for qc in range(4):
                rcp = watn.tile([128, 1], F32, tag="rcp", bufs=8, name=f"rcp_{h}_{qc}")
                nc.vector.reciprocal(out=rcp, in_=utn[:, qc, D : D + 1])
                nc.vector.tensor_scalar_mul(On[:, qc, :], utn[:, qc, 0:D], rcp)
            return On

        tr = ps_tr.tile([128, QSL], BF16, tag="tr", bufs=1, name=f"tr_{hc}")

        def tr_flush(hh, On, tr):
            # PE transposes restore the [d, q] layout the Z matmul needs;
            # head parity picks the output partition half (no DMA shift).
            for qc in range(4):
                nc.tensor.matmul(
                    tr[64 * hh : 64 * hh + 64, 128 * qc : 128 * (qc + 1)],
                    On[:, qc, :],
                    eye_sb,
                    is_transpose=True,
                )

        for hh in range(2):
            h = 2 * hc + hh
            On = head_block(hh, h)
            pending.append(lambda hh=hh, On=On, tr=tr: tr_flush(hh, On, tr))
        pending.append(
            lambda hc=hc, tr=tr: nc.vector.tensor_copy(out=UTp[:, hc, :], in_=tr)
        )

    flush_pending()
    ps_tr.release()
    ps_ut.release()
    ps_st.release()

    # ---- output projection + LayerNorm ----
    zw = tc.alloc_tile_pool(name="zw", bufs=2)
    ps_z = tc.alloc_tile_pool(name="ps_z", bufs=4, space="PSUM")

    for qc in range(4):
        stats = zw.tile([128, 2, 6], F32, tag="stats")
        for ns in range(2):
            ps = ps_z.tile([128, 512], F32, tag="ps_z", name=f"z_{qc}_{ns}")
            for p8 in range(8):
                nc.tensor.matmul(
                    ps,
                    UTp[:, p8, 128 * qc : 128 * (qc + 1)],
                    wo_sb[:, p8, 512 * ns : 512 * (ns + 1)],
                    start=(p8 == 0),
                    stop=False,
                )
            # + bv@Wo + bo folded in as a rank-1 update (ones ⊗ cvec)
            nc.tensor.matmul(
                ps,
                ones_sb,
                cv_sb[0:1, 512 * ns : 512 * (ns + 1)],
                start=False,
                stop=True,
            )
            # stats straight off PSUM (cvec already folded in), in parallel
            # with the PSUM -> SBUF staging on the otherwise-idle ACT engine
            nc.vector.bn_stats(out=stats[:, ns, :], in_=ps)
            nc.scalar.copy(out=Z[:, qc, 512 * ns : 512 * (ns + 1)], in_=ps)
        x = Z[:, qc, :]
        mv = zw.tile([128, 2], F32, tag="mv")
        nc.vector.bn_aggr(out=mv, in_=stats)
        std = zw.tile([128, 1], F32, tag="std")
        nc.scalar.activation(out=std, in_=mv[:, 1:2], func=AF.Sqrt, bias=eps_t)
        rstd = zw.tile([128, 1], F32, tag="rstd")
        nc.vector.reciprocal(out=rstd, in_=std)
        y = zw.tile([128, E], BF16, tag="y")
        yg = zw.tile([128, E], BF16, tag="yg")
        yf = zw.tile([128, E], F32, tag="yf")
        for ns in range(2):
            sl = slice(512 * ns, 512 * (ns + 1))
            nc.vector.tensor_scalar(
                out=y[:, sl],
                in0=x[:, sl],
                scalar1=mv[:, 0:1],
                scalar2=rstd,
                op0=OP.subtract,
                op1=OP.mult,
            )
            nc.vector.tensor_mul(yg[:, sl], y[:, sl], cgb[:, 0, sl])
            if qc < 3:
                nc.gpsimd.tensor_add(yf[:, sl], yg[:, sl], cgb[:, 1, sl])
            else:
                nc.vector.tensor_add(yf[:, sl], yg[:, sl], cgb[:, 1, sl])
            nc.sync.dma_start(
                out=io["out"][128 * qc : 128 * (qc + 1), sl], in_=yf[:, sl]
            )

    ps_z.release()
    zw.release()
    watn.release()
    ps_kv.release()
    stat.release()
    wz.release()
    vp.release()
    qtp.release()
    ktp.release()
    utp.release()
    small.release()
    ctx_lp.__exit__(None, None, None)


def build_bass():
    nc = bacc.Bacc("TRN2", target_bir_lowering=False, debug=False)
    io = {}
    io["qT"] = nc.declare_dram_parameter("qT", [CQ, QSL], BF16, isOutput=False)
    io["kT"] = nc.declare_dram_parameter("kT", [CKV, N2], BF16, isOutput=False)
    io["vT"] = nc.declare_dram_parameter("vT", [CKV, N2], BF16, isOutput=False)
    io["Wq"] = nc.declare_dram_parameter("Wq", [CQ, E], BF16, isOutput=False)
    io["Wk"] = nc.declare_dram_parameter("Wk", [CKV, E], BF16, isOutput=False)
    io["Wv"] = nc.declare_dram_parameter("Wv", [CKV, E], BF16, isOutput=False)
    io["Wo"] = nc.declare_dram_parameter("Wo", [E, E], BF16, isOutput=False)
    io["bq"] = nc.declare_dram_parameter("bq", [E], F32, isOutput=False)
    io["bk"] = nc.declare_dram_parameter("bk", [E], F32, isOutput=False)
    io["ones_bf"] = nc.declare_dram_parameter("ones_bf", [256], BF16, isOutput=False)
    io["eye_bf"] = nc.declare_dram_parameter("eye_bf", [128, 128], BF16, isOutput=False)
    io["cvec"] = nc.declare_dram_parameter("cvec", [E], BF16, isOutput=False)
    io["gamma"] = nc.declare_dram_parameter("gamma", [E], BF16, isOutput=False)
    io["beta"] = nc.declare_dram_parameter("beta", [E], BF16, isOutput=False)
    io["out"] = nc.declare_dram_parameter("out", [QSL, E], F32, isOutput=True)

    with tile.TileContext(nc) as tc:
        _emit(tc, io)
    nc.compile()
    return nc


_NC_CACHE = None


def _get_nc():
    global _NC_CACHE
    if _NC_CACHE is None:
        _NC_CACHE = build_bass()
    return _NC_CACHE


def make_in_maps(query, key, value, Wq, bq, Wk, bk, Wv, bv, Wo, bo, gamma, beta):
    f = np.float32
    cvec = (bv.astype(f) @ Wo.astype(f) + bo.astype(f)).astype(f)
    shared = {
        "Wq": np.ascontiguousarray(Wq).astype(NPBF16),
        "Wk": np.ascontiguousarray(Wk).astype(NPBF16),
        "Wv": np.ascontiguousarray(Wv).astype(NPBF16),
        "Wo": np.ascontiguousarray(Wo).astype(NPBF16),
        "bq": np.ascontiguousarray(bq, f),
        "bk": np.ascontiguousarray(bk, f),
        "cvec": cvec.astype(NPBF16),
        "ones_bf": np.ones(256, NPBF16),
        "eye_bf": np.eye(128, dtype=f).astype(NPBF16),
        "gamma": np.ascontiguousarray(gamma).astype(NPBF16),
        "beta": np.ascontiguousarray(beta).astype(NPBF16),
    }
    kT = [np.ascontiguousarray(key[b].T).astype(NPBF16) for b in range(B)]
    vT = [np.ascontiguousarray(value[b].T).astype(NPBF16) for b in range(B)]
    in_maps = []
    for c in range(NCORES):
        b, r = divmod(c, NCORES // B)
        qT = np.ascontiguousarray(query[b, QSL * r : QSL * (r + 1), :].T).astype(NPBF16)
        m = dict(shared)
        m.update({"qT": qT, "kT": kT[b], "vT": vT[b]})
        in_maps.append(m)
    return in_maps


def kernel(**inputs):
    nc = _get_nc()
    in_maps = make_in_maps(**inputs)
    res = run_bass_kernel_spmd(nc, in_maps, core_ids=list(range(NCORES)))
    out = np.empty((B, N1, E), np.float32)
    for c in range(NCORES):
        b, r = divmod(c, NCORES // B)
        out[b, QSL * r : QSL * (r + 1), :] = res.results[c]["out"]
    return out
